# revision 37
# baseline (speedup 1.0000x reference)
"""Trainium2 Bass kernel for the periodic 9-point diffusion stencil.

Full input:  state [4, 8, 1024, 1024] f32, diffusion_coefficient, dt (scalars).
Full output: [4, 8, 1024, 1024] f32.

Math: out = X + c1*M(X), M = S (x) S - 16 I with S = [1,2,1] periodic and
c1 = scale/12, scale = dc*dt.  The identity term is kept EXACT on the host;
the device computes only the residual D = 0.125*M(X) from an fp8e4m3 copy of
X and returns it as fp8e4m3.  Host: out = state + (2/3)*scale * D.  Errors
(input fp8 rounding through the zero-sum M, output fp8 rounding of D) enter
the result scaled by ~scale, i.e. ~1e-4 relative for the reference's
scale=1e-3 -- far below the 2e-2 gate -- while I/O traffic drops 4x vs f32
(4 MB in + 4 MB out per core; memory-bound regime).

Sharding: 32 independent (b, c) slices of [1024, 1024]; 4 per core, pure data
parallel.  Per slice the 1024 rows live in SBUF as 8 k-tiles of 128 rows
(one 1 MB multi-dim DMA).  Output tiles are 128 rows each, computed from
sliding k-tile pairs (i, i+1) and the wrap pair (7, 0) as fp8 DoubleRow
matmuls (K=256, 0.5 cycles/column): the vertical [1,2,1]/center stencil is a
banded 256x128 weight matrix, horizontal taps are shifted rhs column windows
accumulated in PSUM (column wrap = two 1-wide matmuls).  The output is
written rotated down one row (out row r+1 -> y row r) so each slice's store
is ONE contiguous multi-dim DMA; the host un-rotates in the final add.
PSUM f32 -> fp8 casts rotate across DVE / Act / Pool so no single engine
binds.  For scale > 0.02 an exact-f32 program (the previous baseline) is
used instead.

Measured on the 8-core axon TRN2: see test.py; DMA floor for 8.4 MB/core is
~25 us, PE floor ~21 us.
"""

import numpy as np

N_CORES = 8
S_PER_CORE = 4  # (b,c) slices per core
H = W = 1024
KT = 8  # k-tiles of 128 rows per slice
W0 = 0.125  # base weight: device computes D = W0 * M(X)

_PROGRAMS = {}


def _make_weights(dtype):
    """Banded DoubleRow weights for the pair-interleaved layout.

    A block holds 256 consecutive rows: SBUF partition p, slot t <-> block
    row 2p+t (so each partition's two rows are HBM-adjacent -> 2 KB DMA
    descriptors).  Output slot (m, c) <-> block row 2m+c+1, m in 0..126.
    Weight[p, t, m] = mv(row - out) = mv(2(p-m) + t - c - 1).  m is padded
    to 128 (Ldweights ISA requires full tiles); the m=127 slot computes a
    defined but unused value.  Four tensors (center/side x c=0/1), stored
    [p, t*128 + m], concatenated into one [128, 1024] "wall"."""
    p = np.arange(128)[:, None, None]
    t = np.arange(2)[None, :, None]
    m = np.arange(128)[None, None, :]
    out = {}
    for c in (0, 1):
        d = 2 * (p - m) + t - c - 1
        ctr = np.where(d == 0, -12.0, np.where(np.abs(d) == 1, 2.0, 0.0))
        side = np.where(d == 0, 2.0, np.where(np.abs(d) == 1, 1.0, 0.0))
        out[f"wc{c}"] = (ctr * W0).reshape(128, 256)
        out[f"ws{c}"] = (side * W0).reshape(128, 256)
    wall = np.concatenate(
        [out[n] for n in ("wc0", "ws0", "wc1", "ws1")], axis=1)
    return wall.astype(dtype)


def _build_program(loop_r=1, x_bufs=3, o_bufs=3, ps_bufs=4,
                   in_plan=(2, 1, 1, 1), out_plan=(2, 2, 2, 4),
                   variant="full"):
    from contextlib import ExitStack

    import concourse.bass as bass
    import concourse.tile as tile
    from concourse import bacc, mybir
    from concourse.ap import AP as mkAP
    from concourse.bass_interp import get_hw_module

    f32 = mybir.dt.float32
    fp8 = mybir.dt.float8e4
    DR = mybir.MatmulPerfMode.DoubleRow

    nc = bacc.Bacc("TRN2", target_bir_lowering=False, debug=False,
                   num_devices=N_CORES)
    x = nc.dram_tensor("x", [S_PER_CORE, H, W], fp8, kind="ExternalInput").ap()
    # all 4 weight tensors concatenated: [wac | was | wlc | wls]
    wall = nc.dram_tensor("wall", [128, 1024], fp8, kind="ExternalInput").ap()
    y = nc.dram_tensor("y", [S_PER_CORE, H, W], fp8, kind="ExternalOutput").ap()

    with tile.TileContext(nc) as tc:
        with ExitStack() as ctx:
            consts = ctx.enter_context(tc.tile_pool(name="consts", bufs=1))
            xp = ctx.enter_context(tc.tile_pool(name="x", bufs=x_bufs))
            op = ctx.enter_context(tc.tile_pool(name="o", bufs=o_bufs))
            pp = ctx.enter_context(
                tc.tile_pool(name="ps", bufs=ps_bufs, space="PSUM"))

            # one weight DMA, issued before any input so the PE can
            # preload as soon as possible
            wtall = consts.tile([128, 1024], fp8)
            nc.sync.dma_start(wtall[:], wall[:])
            lhs = {n: wtall[:, 256 * i:256 * (i + 1)].rearrange(
                       "p (t m) -> p t m", t=2)
                   for i, n in enumerate(("wc0", "ws0", "wc1", "ws1"))}

            def stencil_tile(rhs, pt, lc, ls):
                """pt[128, W] (2 PSUM banks) += banded vertical x
                horizontal [1,2,1] taps of the block pair view rhs
                [128, 2, W].  lc/ls: center/side DoubleRow weights
                [128, 2, 128] (partition 127 of pt is pad)."""
                # center taps: first writers of both banks
                nc.tensor.matmul(pt[:, 0:512], lc, rhs[:, :, 0:512],
                                 start=True, stop=False, perf_mode=DR,
                                 skip_group_check=True)
                nc.tensor.matmul(pt[:, 512:1024], lc, rhs[:, :, 512:1024],
                                 start=True, stop=False, perf_mode=DR,
                                 skip_group_check=True)
                # left neighbors: psum[:, j] += Ws @ X[:, j-1]
                nc.tensor.matmul(pt[:, 1:512], ls, rhs[:, :, 0:511],
                                 start=False, stop=False, perf_mode=DR,
                                 skip_group_check=True)
                nc.tensor.matmul(pt[:, 0:1], ls, rhs[:, :, 1023:1024],
                                 start=False, stop=False, perf_mode=DR,
                                 skip_group_check=True)
                nc.tensor.matmul(pt[:, 512:1024], ls, rhs[:, :, 511:1023],
                                 start=False, stop=False, perf_mode=DR,
                                 skip_group_check=True)
                # right neighbors: psum[:, j] += Ws @ X[:, j+1]
                nc.tensor.matmul(pt[:, 0:512], ls, rhs[:, :, 1:513],
                                 start=False, stop=True, perf_mode=DR,
                                 skip_group_check=True)
                nc.tensor.matmul(pt[:, 512:1023], ls, rhs[:, :, 513:1024],
                                 start=False, stop=False, perf_mode=DR,
                                 skip_group_check=True)
                nc.tensor.matmul(pt[:, 1023:1024], ls, rhs[:, :, 0:1],
                                 start=False, stop=True, perf_mode=DR,
                                 skip_group_check=True)

            def body(_i=None):
                if variant in ("dma2k", "dmahw"):
                    # pure-DMA probes: dma2k uses 2KB-contiguous-per-partition
                    # APs (row pairs), dmahw is the 1KB layout with all DMAs
                    # on HWDGE queues
                    for s in range(S_PER_CORE):
                        xt = xp.tile([128, KT * W], fp8, tag="xt")
                        for b in range(4):
                            if variant == "dma2k":
                                view = mkAP(tensor=x[s].tensor,
                                            offset=x[s].offset + b * 256 * W,
                                            ap=[[2 * W, 128], [1, 2 * W]])
                                dst = xt[:, b * 2 * W:(b + 1) * 2 * W]
                            else:
                                view = mkAP(tensor=x[s].tensor,
                                            offset=x[s].offset + b * 256 * W,
                                            ap=[[W, 128], [128 * W, 2], [1, W]])
                                dst = xt[:, b * 2 * W:(b + 1) * 2 * W] \
                                    .rearrange("p (t w) -> p t w", t=2)
                            nc.sync.dma_start(dst, view)
                        for b in range(4):
                            if variant == "dma2k":
                                view = mkAP(tensor=y[s].tensor,
                                            offset=y[s].offset + b * 256 * W,
                                            ap=[[2 * W, 128], [1, 2 * W]])
                                src = xt[:, b * 2 * W:(b + 1) * 2 * W]
                            else:
                                view = mkAP(tensor=y[s].tensor,
                                            offset=y[s].offset + b * 256 * W,
                                            ap=[[W, 128], [128 * W, 2], [1, W]])
                                src = xt[:, b * 2 * W:(b + 1) * 2 * W] \
                                    .rearrange("p (t w) -> p t w", t=2)
                            nc.scalar.dma_start(view, src)
                    return
                for s in range(S_PER_CORE):
                    xs_ = x[s]
                    ys_ = y[s]
                    # pair-interleaved blocks: partition p slot t of block b
                    # <-> image row 253b + 2p + t (rows 0..1014), so every
                    # DMA descriptor covers 2 HBM-adjacent rows (2 KB)
                    xt = xp.tile([128, 4 * 2 * W], fp8, tag="xt")
                    nin = in_plan[s]
                    for h in range(nin):
                        b0, b1 = h * (4 // nin), (h + 1) * (4 // nin)
                        in_view = mkAP(
                            tensor=xs_.tensor,
                            offset=xs_.offset + 253 * b0 * W,
                            ap=[[2 * W, 128], [253 * W, b1 - b0], [1, 2 * W]])
                        nc.sync.dma_start(
                            xt[:, b0 * 2 * W:b1 * 2 * W].rearrange(
                                "p (b u) -> p b u", b=b1 - b0), in_view)
                    x8 = xt[:].rearrange("p (q w) -> p q w", q=8)
                    ot = op.tile([128, 4 * 2 * W], fp8, tag="ot")
                    for b in range(4):
                        if variant == "dma":
                            continue
                        rhs = x8[:, 2 * b:2 * b + 2, :]
                        for c in (0, 1):
                            pt = pp.tile([128, W], f32, tag="pt")
                            stencil_tile(rhs, pt, lhs[f"wc{c}"],
                                         lhs[f"ws{c}"])
                            if variant == "pe":
                                dot = op.tile([1, 2], f32, tag="dummy")
                                nc.vector.tensor_copy(dot[:], pt[0:1, 0:2])
                                continue
                            # GPSIMD may not touch PSUM (BIR verifier):
                            # casts alternate Act / DVE
                            dst = ot[:, (2 * b + c) * W:
                                     (2 * b + c + 1) * W]
                            if c == 0:
                                nc.scalar.copy(dst, pt[:])
                            else:
                                nc.vector.tensor_copy(dst, pt[:])
                    # variants: store from xt (fully initialized) so the
                    # out-DMA never reads uninitialized SBUF (HW parity)
                    src_t = xt if variant in ("dma", "pe") else ot
                    # block b slot (m, c) holds output row 253b + 2m + c + 1;
                    # device covers rows 1..1013, host does 0 and 1014..1023.
                    # Seam rows are written twice with identical values.
                    nsp = out_plan[s]
                    for h in range(nsp):
                        b0, b1 = h * (4 // nsp), (h + 1) * (4 // nsp)
                        out_view = mkAP(
                            tensor=ys_.tensor,
                            offset=ys_.offset + (253 * b0 + 1) * W,
                            ap=[[2 * W, 127], [253 * W, b1 - b0], [1, 2 * W]])
                        # out-DMAs go via the otherwise-idle GPSIMD SWDGE
                        # queue (their wait-on-cast must not stall the SP
                        # input queue; Act is busy casting).  The last
                        # slice's outs use the fast SP/Act HWDGE queues,
                        # idle by then, to shorten the drain.
                        if s == S_PER_CORE - 1:
                            eng_out = nc.sync if h % 2 == 0 else nc.gpsimd
                        else:
                            eng_out = nc.gpsimd
                        eng_out.dma_start(
                            out_view,
                            src_t[0:127, b0 * 2 * W:b1 * 2 * W].rearrange(
                                "p (b u) -> p b u", b=b1 - b0))

            if loop_r == 1:
                body()
            else:
                with tc.For_i(0, loop_r, 1):
                    body()

    nc.compile()
    nc.m = get_hw_module(nc.m)
    return nc


def _get_program(variant="fp8"):
    if variant not in _PROGRAMS:
        if variant == "fp8":
            _PROGRAMS[variant] = _build_program()
        else:
            _PROGRAMS[variant] = _build_program_exact()
    return _PROGRAMS[variant]


def kernel(state, diffusion_coefficient, dt):
    import ml_dtypes
    from concourse.bass_utils import run_bass_kernel_spmd

    state = np.asarray(state)
    in_dtype = state.dtype
    xs32 = np.ascontiguousarray(state, dtype=np.float32).reshape(
        N_CORES * S_PER_CORE, H, W)

    scale = float(np.asarray(diffusion_coefficient, dtype=np.float64)) * \
        float(np.asarray(dt, dtype=np.float64))
    if abs(scale) > 0.02:
        return _kernel_exact(xs32, scale, in_dtype)

    FP8 = ml_dtypes.float8_e4m3
    x8 = np.clip(xs32, -224.0, 224.0).astype(FP8)
    wall = _make_weights(FP8)
    nc = _get_program("fp8")
    in_maps = [
        {"x": x8[k * S_PER_CORE:(k + 1) * S_PER_CORE], "wall": wall}
        for k in range(N_CORES)
    ]
    res = run_bass_kernel_spmd(nc, in_maps, core_ids=list(range(N_CORES)))
    d = np.concatenate([res.results[k]["y"] for k in range(N_CORES)], axis=0)
    out = _postprocess(xs32, d, scale)
    return out.reshape(4, 8, H, W).astype(in_dtype, copy=False)


def _postprocess(xs32, d, scale):
    """out = state + (c1/W0)*D for device rows 1..1013; boundary rows
    (0, 1014..1023 -- vertical wrap region) via an exact f32 stencil on
    the host."""
    g = np.float32(scale / (12.0 * W0))
    out = xs32 + g * d.astype(np.float32)
    R = np.array([0] + list(range(1014, 1024)))
    Xr = xs32[:, R, :]
    Xu = xs32[:, (R - 1) % H, :]
    Xd = xs32[:, (R + 1) % H, :]
    V = Xu + np.float32(2.0) * Xr + Xd
    Hh = np.roll(V, 1, axis=-1) + np.float32(2.0) * V + np.roll(V, -1, axis=-1)
    c1 = np.float32(scale / 12.0)
    c2 = np.float32(1.0 - 4.0 * scale / 3.0)
    out[:, R, :] = c2 * Xr + c1 * Hh
    return out


# ---------------------------------------------------------------------------
# Exact-f32 fallback (previous baseline program) for scale > 0.02.
# ---------------------------------------------------------------------------

ROWS_PER_TILE = 126
N_FULL_TILES = 8


def _build_program_exact(loop_r=1):
    from contextlib import ExitStack

    import concourse.bass as bass
    import concourse.tile as tile
    from concourse import bacc, mybir
    from concourse.bass_interp import get_hw_module

    f32 = mybir.dt.float32
    mult = mybir.AluOpType.mult
    add = mybir.AluOpType.add

    nc = bacc.Bacc("TRN2", target_bir_lowering=False, debug=False,
                   num_devices=N_CORES)
    x = nc.dram_tensor("x", [S_PER_CORE, H, W], f32, kind="ExternalInput").ap()
    w1 = nc.dram_tensor("w1", [128, 128], f32, kind="ExternalInput").ap()
    w2 = nc.dram_tensor("w2", [128, 128], f32, kind="ExternalInput").ap()
    c2v = nc.dram_tensor("c2v", [128, 1], f32, kind="ExternalInput").ap()
    y = nc.dram_tensor("y", [S_PER_CORE, H, W], f32, kind="ExternalOutput").ap()

    with tile.TileContext(nc) as tc:
        with ExitStack() as ctx:
            consts = ctx.enter_context(tc.tile_pool(name="consts", bufs=1))
            xp = ctx.enter_context(tc.tile_pool(name="x", bufs=3))
            op = ctx.enter_context(tc.tile_pool(name="o", bufs=3))
            pp = ctx.enter_context(
                tc.tile_pool(name="ps", bufs=4, space="PSUM"))

            w1t = consts.tile([128, 128], f32)
            nc.sync.dma_start(w1t[:], w1[:])
            w2t = consts.tile([128, 128], f32)
            nc.sync.dma_start(w2t[:], w2[:])
            c2t = consts.tile([128, 1], f32)
            nc.sync.dma_start(c2t[:], c2v[:])

            def stencil_tile(xb, pt, K, base=0):
                l1 = w1t[:K, :K]
                l2 = w2t[:K, :K]
                b = base
                nc.tensor.matmul(pt[:, 0:512], l2, xb[:, b:b + 512],
                                 start=True, stop=False, skip_group_check=True)
                nc.tensor.matmul(pt[:, 512:1024], l2, xb[:, b + 512:b + 1024],
                                 start=True, stop=False, skip_group_check=True)
                nc.tensor.matmul(pt[:, 1:512], l1, xb[:, b:b + 511],
                                 start=False, stop=False, skip_group_check=True)
                nc.tensor.matmul(pt[:, 0:1], l1, xb[:, b + 1023:b + 1024],
                                 start=False, stop=False, skip_group_check=True)
                nc.tensor.matmul(pt[:, 512:1024], l1, xb[:, b + 511:b + 1023],
                                 start=False, stop=False, skip_group_check=True)
                nc.tensor.matmul(pt[:, 0:512], l1, xb[:, b + 1:b + 513],
                                 start=False, stop=True, skip_group_check=True)
                nc.tensor.matmul(pt[:, 512:1023], l1, xb[:, b + 513:b + 1024],
                                 start=False, stop=False, skip_group_check=True)
                nc.tensor.matmul(pt[:, 1023:1024], l1, xb[:, b:b + 1],
                                 start=False, stop=True, skip_group_check=True)

            def body(_i=None):
                from concourse.ap import AP as mkAP
                nblk = 4
                for s in range(S_PER_CORE):
                    xs_ = x[s]
                    ys_ = y[s]
                    n0 = 0
                    for g in range(N_FULL_TILES // nblk):
                        in_view = mkAP(
                            tensor=xs_.tensor,
                            offset=xs_.offset + 126 * n0 * W,
                            ap=[[W, 128], [126 * W, nblk], [1, W]])
                        xt = xp.tile([128, nblk * W], f32, tag="xt")
                        nc.sync.dma_start(
                            xt[:].rearrange("p (n w) -> p n w", n=nblk),
                            in_view)
                        ot = op.tile([128, nblk * W], f32, tag="ot")
                        for b in range(nblk):
                            pt = pp.tile([128, W], f32, tag="pt")
                            stencil_tile(xt[:], pt, 128, base=b * W)
                            nc.vector.scalar_tensor_tensor(
                                ot[:, b * W:(b + 1) * W],
                                xt[:, b * W:(b + 1) * W],
                                c2t[:], pt[:], op0=mult, op1=add)
                        out_view = mkAP(
                            tensor=ys_.tensor,
                            offset=ys_.offset + (126 * n0 + 1) * W,
                            ap=[[W, 126], [126 * W, nblk], [1, W]])
                        nc.scalar.dma_start(
                            out_view,
                            ot[1:127, :].rearrange("p (n w) -> p n w",
                                                   n=nblk))
                        n0 += nblk

                    r0 = N_FULL_TILES * ROWS_PER_TILE + 1  # 1009
                    xt = xp.tile([32, W], f32, tag="xt_last")
                    nc.sync.dma_start(xt[0:16, :], x[s, H - 16:H, :])
                    nc.sync.dma_start(xt[16:32, :], x[s, 0:16, :])
                    pt = pp.tile([32, W], f32, tag="pt")
                    stencil_tile(xt[:], pt, 32)
                    ot = op.tile([32, W], f32, tag="ot")
                    nc.vector.scalar_tensor_tensor(
                        ot[:], xt[:], c2t[0:32, :], pt[:], op0=mult, op1=add)
                    nc.scalar.dma_start(y[s, r0:H, :], ot[1:1 + H - r0, :])
                    nc.scalar.dma_start(y[s, 0:1, :], ot[16:17, :])

            if loop_r == 1:
                body()
            else:
                with tc.For_i(0, loop_r, 1):
                    body()

    nc.compile()
    nc.m = get_hw_module(nc.m)
    return nc


def _kernel_exact(xs32, scale, in_dtype):
    from concourse.bass_utils import run_bass_kernel_spmd

    c1 = scale / 12.0
    c2 = 1.0 - 4.0 * scale / 3.0
    tri = np.zeros((128, 128), dtype=np.float64)
    idx = np.arange(128)
    tri[idx, idx] = 2.0
    tri[idx[:-1], idx[:-1] + 1] = 1.0
    tri[idx[:-1] + 1, idx[:-1]] = 1.0
    nc = _get_program("exact")
    w1 = (c1 * tri).astype(np.float32)
    w2 = (2.0 * c1 * tri).astype(np.float32)
    c2v = np.full((128, 1), c2, dtype=np.float32)
    in_maps = [
        {"x": xs32[k * S_PER_CORE:(k + 1) * S_PER_CORE], "w1": w1, "w2": w2,
         "c2v": c2v}
        for k in range(N_CORES)
    ]
    res = run_bass_kernel_spmd(nc, in_maps, core_ids=list(range(N_CORES)))
    out = np.concatenate([res.results[k]["y"] for k in range(N_CORES)], axis=0)
    return out.reshape(4, 8, H, W).astype(in_dtype, copy=False)


# revision 39
# speedup vs baseline: 1.3688x; 1.3688x over previous
"""Trainium2 Bass kernel for the periodic 9-point diffusion stencil.

Full input:  state [4, 8, 1024, 1024] f32, diffusion_coefficient, dt (scalars).
Full output: [4, 8, 1024, 1024] f32.

Math: out = X + c1*M(X), M = S (x) S - 16 I with S = [1,2,1] periodic and
c1 = scale/12, scale = dc*dt.  The identity term is kept EXACT on the host;
the device computes only the residual D = 0.125*M(X) from an fp8e4m3 copy of
X and returns it as fp8e4m3.  Host: out = state + (2/3)*scale * D.  Errors
(input fp8 rounding through the zero-sum M, output fp8 rounding of D) enter
the result scaled by ~scale, i.e. ~1e-4 relative for the reference's
scale=1e-3 -- far below the 2e-2 gate -- while I/O traffic drops 4x vs f32
(4 MB in + 4 MB out per core; memory-bound regime).

Sharding: 32 independent (b, c) slices of [1024, 1024]; 4 per core, pure data
parallel.  Per slice the 1024 rows live in SBUF as 8 k-tiles of 128 rows
(one 1 MB multi-dim DMA).  Output tiles are 128 rows each, computed from
sliding k-tile pairs (i, i+1) and the wrap pair (7, 0) as fp8 DoubleRow
matmuls (K=256, 0.5 cycles/column): the vertical [1,2,1]/center stencil is a
banded 256x128 weight matrix, horizontal taps are shifted rhs column windows
accumulated in PSUM (column wrap = two 1-wide matmuls).  The output is
written rotated down one row (out row r+1 -> y row r) so each slice's store
is ONE contiguous multi-dim DMA; the host un-rotates in the final add.
PSUM f32 -> fp8 casts rotate across DVE / Act / Pool so no single engine
binds.  For scale > 0.02 an exact-f32 program (the previous baseline) is
used instead.

Measured on the 8-core axon TRN2: see test.py; DMA floor for 8.4 MB/core is
~25 us, PE floor ~21 us.
"""

import numpy as np

N_CORES = 8
S_PER_CORE = 4  # (b,c) slices per core
H = W = 1024
KT = 8  # k-tiles of 128 rows per slice
W0 = 0.125  # base weight: device computes D = W0 * M(X)

_PROGRAMS = {}


def _make_weights(dtype):
    """Banded DoubleRow weights for the pair-interleaved layout.

    A block holds 256 consecutive rows: SBUF partition p, slot t <-> block
    row 2p+t (so each partition's two rows are HBM-adjacent -> 2 KB DMA
    descriptors).  Output slot (m, c) <-> block row 2m+c+1, m in 0..126.
    Weight[p, t, m] = mv(row - out) = mv(2(p-m) + t - c - 1).  m is padded
    to 128 (Ldweights ISA requires full tiles); the m=127 slot computes a
    defined but unused value.  Four tensors (center/side x c=0/1), stored
    [p, t*128 + m], concatenated into one [128, 1024] "wall"."""
    p = np.arange(128)[:, None, None]
    t = np.arange(2)[None, :, None]
    m = np.arange(128)[None, None, :]
    out = {}
    for c in (0, 1):
        d = 2 * (p - m) + t - c - 1
        ctr = np.where(d == 0, -12.0, np.where(np.abs(d) == 1, 2.0, 0.0))
        side = np.where(d == 0, 2.0, np.where(np.abs(d) == 1, 1.0, 0.0))
        out[f"wc{c}"] = (ctr * W0).reshape(128, 256)
        out[f"ws{c}"] = (side * W0).reshape(128, 256)
    wall = np.concatenate(
        [out[n] for n in ("wc0", "ws0", "wc1", "ws1")], axis=1)
    return wall.astype(dtype)


def _build_program(loop_r=1, x_bufs=3, o_bufs=3, ps_bufs=4,
                   in_plan=(2, 1, 1, 1), out_plan=(2, 2, 2, 4),
                   variant="full"):
    from contextlib import ExitStack

    import concourse.bass as bass
    import concourse.tile as tile
    from concourse import bacc, mybir
    from concourse.ap import AP as mkAP
    from concourse.bass_interp import get_hw_module

    f32 = mybir.dt.float32
    fp8 = mybir.dt.float8e4
    DR = mybir.MatmulPerfMode.DoubleRow

    nc = bacc.Bacc("TRN2", target_bir_lowering=False, debug=False,
                   num_devices=N_CORES)
    x = nc.dram_tensor("x", [S_PER_CORE, H, W], fp8, kind="ExternalInput").ap()
    # all 4 weight tensors concatenated: [wac | was | wlc | wls]
    wall = nc.dram_tensor("wall", [128, 1024], fp8, kind="ExternalInput").ap()
    y = nc.dram_tensor("y", [S_PER_CORE, H, W], fp8, kind="ExternalOutput").ap()

    with tile.TileContext(nc) as tc:
        with ExitStack() as ctx:
            consts = ctx.enter_context(tc.tile_pool(name="consts", bufs=1))
            xp = ctx.enter_context(tc.tile_pool(name="x", bufs=x_bufs))
            op = ctx.enter_context(tc.tile_pool(name="o", bufs=o_bufs))
            pp = ctx.enter_context(
                tc.tile_pool(name="ps", bufs=ps_bufs, space="PSUM"))

            # one weight DMA, issued before any input so the PE can
            # preload as soon as possible
            wtall = consts.tile([128, 1024], fp8)
            nc.sync.dma_start(wtall[:], wall[:])
            lhs = {n: wtall[:, 256 * i:256 * (i + 1)].rearrange(
                       "p (t m) -> p t m", t=2)
                   for i, n in enumerate(("wc0", "ws0", "wc1", "ws1"))}

            def stencil_tile(rhs, pt, lc, ls):
                """pt[128, W] (2 PSUM banks) += banded vertical x
                horizontal [1,2,1] taps of the block pair view rhs
                [128, 2, W].  lc/ls: center/side DoubleRow weights
                [128, 2, 128] (partition 127 of pt is pad)."""
                # center taps: first writers of both banks
                nc.tensor.matmul(pt[:, 0:512], lc, rhs[:, :, 0:512],
                                 start=True, stop=False, perf_mode=DR,
                                 skip_group_check=True)
                nc.tensor.matmul(pt[:, 512:1024], lc, rhs[:, :, 512:1024],
                                 start=True, stop=False, perf_mode=DR,
                                 skip_group_check=True)
                # left neighbors: psum[:, j] += Ws @ X[:, j-1]
                nc.tensor.matmul(pt[:, 1:512], ls, rhs[:, :, 0:511],
                                 start=False, stop=False, perf_mode=DR,
                                 skip_group_check=True)
                nc.tensor.matmul(pt[:, 0:1], ls, rhs[:, :, 1023:1024],
                                 start=False, stop=False, perf_mode=DR,
                                 skip_group_check=True)
                nc.tensor.matmul(pt[:, 512:1024], ls, rhs[:, :, 511:1023],
                                 start=False, stop=False, perf_mode=DR,
                                 skip_group_check=True)
                # right neighbors: psum[:, j] += Ws @ X[:, j+1]
                nc.tensor.matmul(pt[:, 0:512], ls, rhs[:, :, 1:513],
                                 start=False, stop=True, perf_mode=DR,
                                 skip_group_check=True)
                nc.tensor.matmul(pt[:, 512:1023], ls, rhs[:, :, 513:1024],
                                 start=False, stop=False, perf_mode=DR,
                                 skip_group_check=True)
                nc.tensor.matmul(pt[:, 1023:1024], ls, rhs[:, :, 0:1],
                                 start=False, stop=True, perf_mode=DR,
                                 skip_group_check=True)

            def body(_i=None):
                if variant in ("dma2k", "dmahw"):
                    # pure-DMA probes: dma2k uses 2KB-contiguous-per-partition
                    # APs (row pairs), dmahw is the 1KB layout with all DMAs
                    # on HWDGE queues
                    for s in range(S_PER_CORE):
                        xt = xp.tile([128, KT * W], fp8, tag="xt")
                        for b in range(4):
                            if variant == "dma2k":
                                view = mkAP(tensor=x[s].tensor,
                                            offset=x[s].offset + b * 256 * W,
                                            ap=[[2 * W, 128], [1, 2 * W]])
                                dst = xt[:, b * 2 * W:(b + 1) * 2 * W]
                            else:
                                view = mkAP(tensor=x[s].tensor,
                                            offset=x[s].offset + b * 256 * W,
                                            ap=[[W, 128], [128 * W, 2], [1, W]])
                                dst = xt[:, b * 2 * W:(b + 1) * 2 * W] \
                                    .rearrange("p (t w) -> p t w", t=2)
                            nc.sync.dma_start(dst, view)
                        for b in range(4):
                            if variant == "dma2k":
                                view = mkAP(tensor=y[s].tensor,
                                            offset=y[s].offset + b * 256 * W,
                                            ap=[[2 * W, 128], [1, 2 * W]])
                                src = xt[:, b * 2 * W:(b + 1) * 2 * W]
                            else:
                                view = mkAP(tensor=y[s].tensor,
                                            offset=y[s].offset + b * 256 * W,
                                            ap=[[W, 128], [128 * W, 2], [1, W]])
                                src = xt[:, b * 2 * W:(b + 1) * 2 * W] \
                                    .rearrange("p (t w) -> p t w", t=2)
                            nc.scalar.dma_start(view, src)
                    return
                for s in range(S_PER_CORE):
                    xs_ = x[s]
                    ys_ = y[s]
                    # pair-interleaved blocks: partition p slot t of block b
                    # <-> image row 253b + 2p + t (rows 0..1014).  One 2D
                    # DMA per block: partition stride (2W) == contiguous run
                    # (2W), so the whole 256KB coalesces into one flat HBM
                    # range (strided/3D APs measured 3-5x slower on HW).
                    xt = xp.tile([128, 4 * 2 * W], fp8, tag="xt")
                    for b in range(4):
                        in_view = mkAP(
                            tensor=xs_.tensor,
                            offset=xs_.offset + 253 * b * W,
                            ap=[[2 * W, 128], [1, 2 * W]])
                        nc.sync.dma_start(
                            xt[:, b * 2 * W:(b + 1) * 2 * W], in_view)
                    x8 = xt[:].rearrange("p (q w) -> p q w", q=8)
                    ot = op.tile([128, 4 * 2 * W], fp8, tag="ot")
                    for b in range(4):
                        if variant == "dma":
                            continue
                        rhs = x8[:, 2 * b:2 * b + 2, :]
                        for c in (0, 1):
                            pt = pp.tile([128, W], f32, tag="pt")
                            stencil_tile(rhs, pt, lhs[f"wc{c}"],
                                         lhs[f"ws{c}"])
                            if variant == "pe":
                                dot = op.tile([1, 2], f32, tag="dummy")
                                nc.vector.tensor_copy(dot[:], pt[0:1, 0:2])
                                continue
                            # GPSIMD may not touch PSUM (BIR verifier):
                            # casts alternate Act / DVE
                            dst = ot[:, (2 * b + c) * W:
                                     (2 * b + c + 1) * W]
                            if c == 0:
                                nc.scalar.copy(dst, pt[:])
                            else:
                                nc.vector.tensor_copy(dst, pt[:])
                    # variants: store from xt (fully initialized) so the
                    # out-DMA never reads uninitialized SBUF (HW parity)
                    src_t = xt if variant in ("dma", "pe") else ot
                    # block b slot (m, c) holds output row 253b + 2m + c + 1;
                    # device covers rows 1..1013, host does 0 and 1014..1023.
                    # Seam rows are written twice with identical values.
                    # Same flat-coalescible 2D AP per block as the input.
                    for b in range(4):
                        out_view = mkAP(
                            tensor=ys_.tensor,
                            offset=ys_.offset + (253 * b + 1) * W,
                            ap=[[2 * W, 127], [1, 2 * W]])
                        # out-DMAs go via the otherwise-idle GPSIMD SWDGE
                        # queue (their wait-on-cast must not stall the SP
                        # input queue; Act is busy casting).  The last
                        # slice's outs use the fast SP/Act HWDGE queues,
                        # idle by then, to shorten the drain.
                        if s == S_PER_CORE - 1:
                            eng_out = nc.sync if b % 2 == 0 else nc.gpsimd
                        else:
                            eng_out = nc.gpsimd
                        eng_out.dma_start(
                            out_view,
                            src_t[0:127, b * 2 * W:(b + 1) * 2 * W])

            if loop_r == 1:
                body()
            else:
                with tc.For_i(0, loop_r, 1):
                    body()

    nc.compile()
    nc.m = get_hw_module(nc.m)
    return nc


def _get_program(variant="fp8"):
    if variant not in _PROGRAMS:
        if variant == "fp8":
            _PROGRAMS[variant] = _build_program()
        else:
            _PROGRAMS[variant] = _build_program_exact()
    return _PROGRAMS[variant]


def kernel(state, diffusion_coefficient, dt):
    import ml_dtypes
    from concourse.bass_utils import run_bass_kernel_spmd

    state = np.asarray(state)
    in_dtype = state.dtype
    xs32 = np.ascontiguousarray(state, dtype=np.float32).reshape(
        N_CORES * S_PER_CORE, H, W)

    scale = float(np.asarray(diffusion_coefficient, dtype=np.float64)) * \
        float(np.asarray(dt, dtype=np.float64))
    if abs(scale) > 0.02:
        return _kernel_exact(xs32, scale, in_dtype)

    FP8 = ml_dtypes.float8_e4m3
    x8 = np.clip(xs32, -224.0, 224.0).astype(FP8)
    wall = _make_weights(FP8)
    nc = _get_program("fp8")
    in_maps = [
        {"x": x8[k * S_PER_CORE:(k + 1) * S_PER_CORE], "wall": wall}
        for k in range(N_CORES)
    ]
    res = run_bass_kernel_spmd(nc, in_maps, core_ids=list(range(N_CORES)))
    d = np.concatenate([res.results[k]["y"] for k in range(N_CORES)], axis=0)
    out = _postprocess(xs32, d, scale)
    return out.reshape(4, 8, H, W).astype(in_dtype, copy=False)


def _postprocess(xs32, d, scale):
    """out = state + (c1/W0)*D for device rows 1..1013; boundary rows
    (0, 1014..1023 -- vertical wrap region) via an exact f32 stencil on
    the host."""
    g = np.float32(scale / (12.0 * W0))
    out = xs32 + g * d.astype(np.float32)
    R = np.array([0] + list(range(1014, 1024)))
    Xr = xs32[:, R, :]
    Xu = xs32[:, (R - 1) % H, :]
    Xd = xs32[:, (R + 1) % H, :]
    V = Xu + np.float32(2.0) * Xr + Xd
    Hh = np.roll(V, 1, axis=-1) + np.float32(2.0) * V + np.roll(V, -1, axis=-1)
    c1 = np.float32(scale / 12.0)
    c2 = np.float32(1.0 - 4.0 * scale / 3.0)
    out[:, R, :] = c2 * Xr + c1 * Hh
    return out


# ---------------------------------------------------------------------------
# Exact-f32 fallback (previous baseline program) for scale > 0.02.
# ---------------------------------------------------------------------------

ROWS_PER_TILE = 126
N_FULL_TILES = 8


def _build_program_exact(loop_r=1):
    from contextlib import ExitStack

    import concourse.bass as bass
    import concourse.tile as tile
    from concourse import bacc, mybir
    from concourse.bass_interp import get_hw_module

    f32 = mybir.dt.float32
    mult = mybir.AluOpType.mult
    add = mybir.AluOpType.add

    nc = bacc.Bacc("TRN2", target_bir_lowering=False, debug=False,
                   num_devices=N_CORES)
    x = nc.dram_tensor("x", [S_PER_CORE, H, W], f32, kind="ExternalInput").ap()
    w1 = nc.dram_tensor("w1", [128, 128], f32, kind="ExternalInput").ap()
    w2 = nc.dram_tensor("w2", [128, 128], f32, kind="ExternalInput").ap()
    c2v = nc.dram_tensor("c2v", [128, 1], f32, kind="ExternalInput").ap()
    y = nc.dram_tensor("y", [S_PER_CORE, H, W], f32, kind="ExternalOutput").ap()

    with tile.TileContext(nc) as tc:
        with ExitStack() as ctx:
            consts = ctx.enter_context(tc.tile_pool(name="consts", bufs=1))
            xp = ctx.enter_context(tc.tile_pool(name="x", bufs=3))
            op = ctx.enter_context(tc.tile_pool(name="o", bufs=3))
            pp = ctx.enter_context(
                tc.tile_pool(name="ps", bufs=4, space="PSUM"))

            w1t = consts.tile([128, 128], f32)
            nc.sync.dma_start(w1t[:], w1[:])
            w2t = consts.tile([128, 128], f32)
            nc.sync.dma_start(w2t[:], w2[:])
            c2t = consts.tile([128, 1], f32)
            nc.sync.dma_start(c2t[:], c2v[:])

            def stencil_tile(xb, pt, K, base=0):
                l1 = w1t[:K, :K]
                l2 = w2t[:K, :K]
                b = base
                nc.tensor.matmul(pt[:, 0:512], l2, xb[:, b:b + 512],
                                 start=True, stop=False, skip_group_check=True)
                nc.tensor.matmul(pt[:, 512:1024], l2, xb[:, b + 512:b + 1024],
                                 start=True, stop=False, skip_group_check=True)
                nc.tensor.matmul(pt[:, 1:512], l1, xb[:, b:b + 511],
                                 start=False, stop=False, skip_group_check=True)
                nc.tensor.matmul(pt[:, 0:1], l1, xb[:, b + 1023:b + 1024],
                                 start=False, stop=False, skip_group_check=True)
                nc.tensor.matmul(pt[:, 512:1024], l1, xb[:, b + 511:b + 1023],
                                 start=False, stop=False, skip_group_check=True)
                nc.tensor.matmul(pt[:, 0:512], l1, xb[:, b + 1:b + 513],
                                 start=False, stop=True, skip_group_check=True)
                nc.tensor.matmul(pt[:, 512:1023], l1, xb[:, b + 513:b + 1024],
                                 start=False, stop=False, skip_group_check=True)
                nc.tensor.matmul(pt[:, 1023:1024], l1, xb[:, b:b + 1],
                                 start=False, stop=True, skip_group_check=True)

            def body(_i=None):
                from concourse.ap import AP as mkAP
                nblk = 4
                for s in range(S_PER_CORE):
                    xs_ = x[s]
                    ys_ = y[s]
                    n0 = 0
                    for g in range(N_FULL_TILES // nblk):
                        in_view = mkAP(
                            tensor=xs_.tensor,
                            offset=xs_.offset + 126 * n0 * W,
                            ap=[[W, 128], [126 * W, nblk], [1, W]])
                        xt = xp.tile([128, nblk * W], f32, tag="xt")
                        nc.sync.dma_start(
                            xt[:].rearrange("p (n w) -> p n w", n=nblk),
                            in_view)
                        ot = op.tile([128, nblk * W], f32, tag="ot")
                        for b in range(nblk):
                            pt = pp.tile([128, W], f32, tag="pt")
                            stencil_tile(xt[:], pt, 128, base=b * W)
                            nc.vector.scalar_tensor_tensor(
                                ot[:, b * W:(b + 1) * W],
                                xt[:, b * W:(b + 1) * W],
                                c2t[:], pt[:], op0=mult, op1=add)
                        out_view = mkAP(
                            tensor=ys_.tensor,
                            offset=ys_.offset + (126 * n0 + 1) * W,
                            ap=[[W, 126], [126 * W, nblk], [1, W]])
                        nc.scalar.dma_start(
                            out_view,
                            ot[1:127, :].rearrange("p (n w) -> p n w",
                                                   n=nblk))
                        n0 += nblk

                    r0 = N_FULL_TILES * ROWS_PER_TILE + 1  # 1009
                    xt = xp.tile([32, W], f32, tag="xt_last")
                    nc.sync.dma_start(xt[0:16, :], x[s, H - 16:H, :])
                    nc.sync.dma_start(xt[16:32, :], x[s, 0:16, :])
                    pt = pp.tile([32, W], f32, tag="pt")
                    stencil_tile(xt[:], pt, 32)
                    ot = op.tile([32, W], f32, tag="ot")
                    nc.vector.scalar_tensor_tensor(
                        ot[:], xt[:], c2t[0:32, :], pt[:], op0=mult, op1=add)
                    nc.scalar.dma_start(y[s, r0:H, :], ot[1:1 + H - r0, :])
                    nc.scalar.dma_start(y[s, 0:1, :], ot[16:17, :])

            if loop_r == 1:
                body()
            else:
                with tc.For_i(0, loop_r, 1):
                    body()

    nc.compile()
    nc.m = get_hw_module(nc.m)
    return nc


def _kernel_exact(xs32, scale, in_dtype):
    from concourse.bass_utils import run_bass_kernel_spmd

    c1 = scale / 12.0
    c2 = 1.0 - 4.0 * scale / 3.0
    tri = np.zeros((128, 128), dtype=np.float64)
    idx = np.arange(128)
    tri[idx, idx] = 2.0
    tri[idx[:-1], idx[:-1] + 1] = 1.0
    tri[idx[:-1] + 1, idx[:-1]] = 1.0
    nc = _get_program("exact")
    w1 = (c1 * tri).astype(np.float32)
    w2 = (2.0 * c1 * tri).astype(np.float32)
    c2v = np.full((128, 1), c2, dtype=np.float32)
    in_maps = [
        {"x": xs32[k * S_PER_CORE:(k + 1) * S_PER_CORE], "w1": w1, "w2": w2,
         "c2v": c2v}
        for k in range(N_CORES)
    ]
    res = run_bass_kernel_spmd(nc, in_maps, core_ids=list(range(N_CORES)))
    out = np.concatenate([res.results[k]["y"] for k in range(N_CORES)], axis=0)
    return out.reshape(4, 8, H, W).astype(in_dtype, copy=False)


# revision 43
# speedup vs baseline: 2.3329x; 1.7044x over previous
"""Trainium2 Bass kernel for the periodic 9-point diffusion stencil.

Full input:  state [4, 8, 1024, 1024] f32, diffusion_coefficient, dt (scalars).
Full output: [4, 8, 1024, 1024] f32.

Math: out = X + c1*M(X), M = S (x) S - 16 I with S = [1,2,1] periodic and
c1 = scale/12, scale = dc*dt.  The identity term is kept EXACT on the host;
the device computes only the residual D = 0.125*M(X) from an fp8e4m3 copy of
X and returns it as fp8e4m3.  Host: out = state + (2/3)*scale * D.  Errors
(input fp8 rounding through the zero-sum M, output fp8 rounding of D) enter
the result scaled by ~scale, i.e. ~1e-4 relative for the reference's
scale=1e-3 -- far below the 2e-2 gate -- while I/O traffic drops 4x vs f32
(4 MB in + 4 MB out per core; memory-bound regime).

Sharding: 32 independent (b, c) slices of [1024, 1024]; 4 per core, pure data
parallel.  Per slice the 1024 rows live in SBUF as 8 k-tiles of 128 rows
(one 1 MB multi-dim DMA).  Output tiles are 128 rows each, computed from
sliding k-tile pairs (i, i+1) and the wrap pair (7, 0) as fp8 DoubleRow
matmuls (K=256, 0.5 cycles/column): the vertical [1,2,1]/center stencil is a
banded 256x128 weight matrix, horizontal taps are shifted rhs column windows
accumulated in PSUM (column wrap = two 1-wide matmuls).  The output is
written rotated down one row (out row r+1 -> y row r) so each slice's store
is ONE contiguous multi-dim DMA; the host un-rotates in the final add.
PSUM f32 -> fp8 casts rotate across DVE / Act / Pool so no single engine
binds.  For scale > 0.02 an exact-f32 program (the previous baseline) is
used instead.

Measured on the 8-core axon TRN2: see test.py; DMA floor for 8.4 MB/core is
~25 us, PE floor ~21 us.
"""

import numpy as np

N_CORES = 8
S_PER_CORE = 4  # (b,c) slices per core
H = W = 1024
KT = 8  # k-tiles of 128 rows per slice
W0 = 0.125  # base weight: device computes D = W0 * M(X)

_PROGRAMS = {}


def _make_weights(dtype):
    """Banded DoubleRow weights for the pair-interleaved layout.

    A block holds 256 consecutive rows: SBUF partition p, slot t <-> block
    row 2p+t (so each partition's two rows are HBM-adjacent -> 2 KB DMA
    descriptors).  Output slot (m, c) <-> block row 2m+c+1, m in 0..126.
    Weight[p, t, m] = mv(row - out) = mv(2(p-m) + t - c).  The first (m=0,
    c=0) and last (m=127, c=1) output rows of a block miss one tap (it
    falls outside the block); those rows are recomputed on the host.
    Four tensors (center/side x c=0/1), stored [p, t*128 + m], concatenated
    into one [128, 1024] "wall"."""
    p = np.arange(128)[:, None, None]
    t = np.arange(2)[None, :, None]
    m = np.arange(128)[None, None, :]
    out = {}
    for c in (0, 1):
        d = 2 * (p - m) + t - c
        ctr = np.where(d == 0, -12.0, np.where(np.abs(d) == 1, 2.0, 0.0))
        side = np.where(d == 0, 2.0, np.where(np.abs(d) == 1, 1.0, 0.0))
        out[f"wc{c}"] = (ctr * W0).reshape(128, 256)
        out[f"ws{c}"] = (side * W0).reshape(128, 256)
    wall = np.concatenate(
        [out[n] for n in ("wc0", "ws0", "wc1", "ws1")], axis=1)
    return wall.astype(dtype)


def _build_program(loop_r=1, x_bufs=3, o_bufs=3, ps_bufs=4,
                   in_plan=(2, 1, 1, 1), out_plan=(2, 2, 2, 4),
                   variant="full"):
    from contextlib import ExitStack

    import concourse.bass as bass
    import concourse.tile as tile
    from concourse import bacc, mybir
    from concourse.ap import AP as mkAP
    from concourse.bass_interp import get_hw_module

    f32 = mybir.dt.float32
    fp8 = mybir.dt.float8e4
    DR = mybir.MatmulPerfMode.DoubleRow

    nc = bacc.Bacc("TRN2", target_bir_lowering=False, debug=False,
                   num_devices=N_CORES)
    x = nc.dram_tensor("x", [S_PER_CORE, H, W], fp8, kind="ExternalInput").ap()
    # all 4 weight tensors concatenated: [wac | was | wlc | wls]
    wall = nc.dram_tensor("wall", [128, 1024], fp8, kind="ExternalInput").ap()
    y = nc.dram_tensor("y", [S_PER_CORE, H, W], fp8, kind="ExternalOutput").ap()

    with tile.TileContext(nc) as tc:
        with ExitStack() as ctx:
            consts = ctx.enter_context(tc.tile_pool(name="consts", bufs=1))
            xp = ctx.enter_context(tc.tile_pool(name="x", bufs=x_bufs))
            op = ctx.enter_context(tc.tile_pool(name="o", bufs=o_bufs))
            pp = ctx.enter_context(
                tc.tile_pool(name="ps", bufs=ps_bufs, space="PSUM"))

            # one weight DMA, issued before any input so the PE can
            # preload as soon as possible
            wtall = consts.tile([128, 1024], fp8)
            nc.sync.dma_start(wtall[:], wall[:])
            lhs = {n: wtall[:, 256 * i:256 * (i + 1)].rearrange(
                       "p (t m) -> p t m", t=2)
                   for i, n in enumerate(("wc0", "ws0", "wc1", "ws1"))}

            def stencil_tile(rhs, pt, lc, ls):
                """pt[128, W] (2 PSUM banks) += banded vertical x
                horizontal [1,2,1] taps of the block pair view rhs
                [128, 2, W].  lc/ls: center/side DoubleRow weights
                [128, 2, 128] (partition 127 of pt is pad)."""
                # center taps: first writers of both banks
                nc.tensor.matmul(pt[:, 0:512], lc, rhs[:, :, 0:512],
                                 start=True, stop=False, perf_mode=DR,
                                 skip_group_check=True)
                nc.tensor.matmul(pt[:, 512:1024], lc, rhs[:, :, 512:1024],
                                 start=True, stop=False, perf_mode=DR,
                                 skip_group_check=True)
                # left neighbors: psum[:, j] += Ws @ X[:, j-1]
                nc.tensor.matmul(pt[:, 1:512], ls, rhs[:, :, 0:511],
                                 start=False, stop=False, perf_mode=DR,
                                 skip_group_check=True)
                nc.tensor.matmul(pt[:, 0:1], ls, rhs[:, :, 1023:1024],
                                 start=False, stop=False, perf_mode=DR,
                                 skip_group_check=True)
                nc.tensor.matmul(pt[:, 512:1024], ls, rhs[:, :, 511:1023],
                                 start=False, stop=False, perf_mode=DR,
                                 skip_group_check=True)
                # right neighbors: psum[:, j] += Ws @ X[:, j+1]
                nc.tensor.matmul(pt[:, 0:512], ls, rhs[:, :, 1:513],
                                 start=False, stop=True, perf_mode=DR,
                                 skip_group_check=True)
                nc.tensor.matmul(pt[:, 512:1023], ls, rhs[:, :, 513:1024],
                                 start=False, stop=False, perf_mode=DR,
                                 skip_group_check=True)
                nc.tensor.matmul(pt[:, 1023:1024], ls, rhs[:, :, 0:1],
                                 start=False, stop=True, perf_mode=DR,
                                 skip_group_check=True)

            def body(_i=None):
                if variant in ("dma2k", "dmahw"):
                    # pure-DMA probes: dma2k uses 2KB-contiguous-per-partition
                    # APs (row pairs), dmahw is the 1KB layout with all DMAs
                    # on HWDGE queues
                    for s in range(S_PER_CORE):
                        xt = xp.tile([128, KT * W], fp8, tag="xt")
                        for b in range(4):
                            if variant == "dma2k":
                                view = mkAP(tensor=x[s].tensor,
                                            offset=x[s].offset + b * 256 * W,
                                            ap=[[2 * W, 128], [1, 2 * W]])
                                dst = xt[:, b * 2 * W:(b + 1) * 2 * W]
                            else:
                                view = mkAP(tensor=x[s].tensor,
                                            offset=x[s].offset + b * 256 * W,
                                            ap=[[W, 128], [128 * W, 2], [1, W]])
                                dst = xt[:, b * 2 * W:(b + 1) * 2 * W] \
                                    .rearrange("p (t w) -> p t w", t=2)
                            nc.sync.dma_start(dst, view)
                        for b in range(4):
                            if variant == "dma2k":
                                view = mkAP(tensor=y[s].tensor,
                                            offset=y[s].offset + b * 256 * W,
                                            ap=[[2 * W, 128], [1, 2 * W]])
                                src = xt[:, b * 2 * W:(b + 1) * 2 * W]
                            else:
                                view = mkAP(tensor=y[s].tensor,
                                            offset=y[s].offset + b * 256 * W,
                                            ap=[[W, 128], [128 * W, 2], [1, W]])
                                src = xt[:, b * 2 * W:(b + 1) * 2 * W] \
                                    .rearrange("p (t w) -> p t w", t=2)
                            nc.scalar.dma_start(view, src)
                    return
                for s in range(S_PER_CORE):
                    xs_ = x[s]
                    ys_ = y[s]
                    # pair-interleaved blocks: partition p slot t of block b
                    # <-> image row 256b + 2p + t.  One 2D DMA per block:
                    # partition stride (2W) == contiguous run (2W), so the
                    # whole 256KB coalesces into one flat aligned HBM range
                    # (strided/3D/odd-offset APs measured 3-5x slower on HW).
                    xt = xp.tile([128, 4 * 2 * W], fp8, tag="xt")
                    for b in range(4):
                        in_view = mkAP(
                            tensor=xs_.tensor,
                            offset=xs_.offset + 256 * b * W,
                            ap=[[2 * W, 128], [1, 2 * W]])
                        nc.sync.dma_start(
                            xt[:, b * 2 * W:(b + 1) * 2 * W], in_view)
                    x8 = xt[:].rearrange("p (q w) -> p q w", q=8)
                    ot = op.tile([128, 4 * 2 * W], fp8, tag="ot")
                    for b in range(4):
                        if variant == "dma":
                            continue
                        rhs = x8[:, 2 * b:2 * b + 2, :]
                        for c in (0, 1):
                            pt = pp.tile([128, W], f32, tag="pt")
                            stencil_tile(rhs, pt, lhs[f"wc{c}"],
                                         lhs[f"ws{c}"])
                            if variant == "pe":
                                dot = op.tile([1, 2], f32, tag="dummy")
                                nc.vector.tensor_copy(dot[:], pt[0:1, 0:2])
                                continue
                            # GPSIMD may not touch PSUM (BIR verifier):
                            # casts alternate Act / DVE
                            dst = ot[:, (2 * b + c) * W:
                                     (2 * b + c + 1) * W]
                            if c == 0:
                                nc.scalar.copy(dst, pt[:])
                            else:
                                nc.vector.tensor_copy(dst, pt[:])
                    # variants: store from xt (fully initialized) so the
                    # out-DMA never reads uninitialized SBUF (HW parity)
                    src_t = xt if variant in ("dma", "pe") else ot
                    # block b slot (m, c) holds output row 256b + 2m + c;
                    # the block's edge rows 256b and 256b+255 miss one tap
                    # and are recomputed on the host along with the vertical
                    # wrap.  Same flat 2D AP per block as the input.
                    for b in range(4):
                        out_view = mkAP(
                            tensor=ys_.tensor,
                            offset=ys_.offset + 256 * b * W,
                            ap=[[2 * W, 128], [1, 2 * W]])
                        # out-DMAs go via the otherwise-idle GPSIMD SWDGE
                        # queue (their wait-on-cast must not stall the SP
                        # input queue; Act is busy casting).  The last
                        # slice's outs use the fast SP/Act HWDGE queues,
                        # idle by then, to shorten the drain.
                        if s == S_PER_CORE - 1:
                            eng_out = nc.sync if b % 2 == 0 else nc.gpsimd
                        else:
                            eng_out = nc.gpsimd
                        eng_out.dma_start(
                            out_view,
                            src_t[:, b * 2 * W:(b + 1) * 2 * W])

            if loop_r == 1:
                body()
            else:
                with tc.For_i(0, loop_r, 1):
                    body()

    nc.compile()
    nc.m = get_hw_module(nc.m)
    return nc


def _get_program(variant="fp8"):
    if variant not in _PROGRAMS:
        if variant == "fp8":
            _PROGRAMS[variant] = _build_program()
        else:
            _PROGRAMS[variant] = _build_program_exact()
    return _PROGRAMS[variant]


def kernel(state, diffusion_coefficient, dt):
    import ml_dtypes
    from concourse.bass_utils import run_bass_kernel_spmd

    state = np.asarray(state)
    in_dtype = state.dtype
    xs32 = np.ascontiguousarray(state, dtype=np.float32).reshape(
        N_CORES * S_PER_CORE, H, W)

    scale = float(np.asarray(diffusion_coefficient, dtype=np.float64)) * \
        float(np.asarray(dt, dtype=np.float64))
    if abs(scale) > 0.02:
        return _kernel_exact(xs32, scale, in_dtype)

    FP8 = ml_dtypes.float8_e4m3
    x8 = np.clip(xs32, -224.0, 224.0).astype(FP8)
    wall = _make_weights(FP8)
    nc = _get_program("fp8")
    in_maps = [
        {"x": x8[k * S_PER_CORE:(k + 1) * S_PER_CORE], "wall": wall}
        for k in range(N_CORES)
    ]
    res = run_bass_kernel_spmd(nc, in_maps, core_ids=list(range(N_CORES)))
    d = np.concatenate([res.results[k]["y"] for k in range(N_CORES)], axis=0)
    out = _postprocess(xs32, d, scale)
    return out.reshape(4, 8, H, W).astype(in_dtype, copy=False)


def _postprocess(xs32, d, scale):
    """out = state + (c1/W0)*D for device rows; block-edge rows (one tap
    missing on device) and the vertical wrap rows via an exact f32 stencil
    on the host."""
    g = np.float32(scale / (12.0 * W0))
    out = xs32 + g * d.astype(np.float32)
    R = np.array([0, 255, 256, 511, 512, 767, 768, 1023])
    Xr = xs32[:, R, :]
    Xu = xs32[:, (R - 1) % H, :]
    Xd = xs32[:, (R + 1) % H, :]
    V = Xu + np.float32(2.0) * Xr + Xd
    Hh = np.roll(V, 1, axis=-1) + np.float32(2.0) * V + np.roll(V, -1, axis=-1)
    c1 = np.float32(scale / 12.0)
    c2 = np.float32(1.0 - 4.0 * scale / 3.0)
    out[:, R, :] = c2 * Xr + c1 * Hh
    return out


# ---------------------------------------------------------------------------
# Exact-f32 fallback (previous baseline program) for scale > 0.02.
# ---------------------------------------------------------------------------

ROWS_PER_TILE = 126
N_FULL_TILES = 8


def _build_program_exact(loop_r=1):
    from contextlib import ExitStack

    import concourse.bass as bass
    import concourse.tile as tile
    from concourse import bacc, mybir
    from concourse.bass_interp import get_hw_module

    f32 = mybir.dt.float32
    mult = mybir.AluOpType.mult
    add = mybir.AluOpType.add

    nc = bacc.Bacc("TRN2", target_bir_lowering=False, debug=False,
                   num_devices=N_CORES)
    x = nc.dram_tensor("x", [S_PER_CORE, H, W], f32, kind="ExternalInput").ap()
    w1 = nc.dram_tensor("w1", [128, 128], f32, kind="ExternalInput").ap()
    w2 = nc.dram_tensor("w2", [128, 128], f32, kind="ExternalInput").ap()
    c2v = nc.dram_tensor("c2v", [128, 1], f32, kind="ExternalInput").ap()
    y = nc.dram_tensor("y", [S_PER_CORE, H, W], f32, kind="ExternalOutput").ap()

    with tile.TileContext(nc) as tc:
        with ExitStack() as ctx:
            consts = ctx.enter_context(tc.tile_pool(name="consts", bufs=1))
            xp = ctx.enter_context(tc.tile_pool(name="x", bufs=3))
            op = ctx.enter_context(tc.tile_pool(name="o", bufs=3))
            pp = ctx.enter_context(
                tc.tile_pool(name="ps", bufs=4, space="PSUM"))

            w1t = consts.tile([128, 128], f32)
            nc.sync.dma_start(w1t[:], w1[:])
            w2t = consts.tile([128, 128], f32)
            nc.sync.dma_start(w2t[:], w2[:])
            c2t = consts.tile([128, 1], f32)
            nc.sync.dma_start(c2t[:], c2v[:])

            def stencil_tile(xb, pt, K, base=0):
                l1 = w1t[:K, :K]
                l2 = w2t[:K, :K]
                b = base
                nc.tensor.matmul(pt[:, 0:512], l2, xb[:, b:b + 512],
                                 start=True, stop=False, skip_group_check=True)
                nc.tensor.matmul(pt[:, 512:1024], l2, xb[:, b + 512:b + 1024],
                                 start=True, stop=False, skip_group_check=True)
                nc.tensor.matmul(pt[:, 1:512], l1, xb[:, b:b + 511],
                                 start=False, stop=False, skip_group_check=True)
                nc.tensor.matmul(pt[:, 0:1], l1, xb[:, b + 1023:b + 1024],
                                 start=False, stop=False, skip_group_check=True)
                nc.tensor.matmul(pt[:, 512:1024], l1, xb[:, b + 511:b + 1023],
                                 start=False, stop=False, skip_group_check=True)
                nc.tensor.matmul(pt[:, 0:512], l1, xb[:, b + 1:b + 513],
                                 start=False, stop=True, skip_group_check=True)
                nc.tensor.matmul(pt[:, 512:1023], l1, xb[:, b + 513:b + 1024],
                                 start=False, stop=False, skip_group_check=True)
                nc.tensor.matmul(pt[:, 1023:1024], l1, xb[:, b:b + 1],
                                 start=False, stop=True, skip_group_check=True)

            def body(_i=None):
                from concourse.ap import AP as mkAP
                nblk = 4
                for s in range(S_PER_CORE):
                    xs_ = x[s]
                    ys_ = y[s]
                    n0 = 0
                    for g in range(N_FULL_TILES // nblk):
                        in_view = mkAP(
                            tensor=xs_.tensor,
                            offset=xs_.offset + 126 * n0 * W,
                            ap=[[W, 128], [126 * W, nblk], [1, W]])
                        xt = xp.tile([128, nblk * W], f32, tag="xt")
                        nc.sync.dma_start(
                            xt[:].rearrange("p (n w) -> p n w", n=nblk),
                            in_view)
                        ot = op.tile([128, nblk * W], f32, tag="ot")
                        for b in range(nblk):
                            pt = pp.tile([128, W], f32, tag="pt")
                            stencil_tile(xt[:], pt, 128, base=b * W)
                            nc.vector.scalar_tensor_tensor(
                                ot[:, b * W:(b + 1) * W],
                                xt[:, b * W:(b + 1) * W],
                                c2t[:], pt[:], op0=mult, op1=add)
                        out_view = mkAP(
                            tensor=ys_.tensor,
                            offset=ys_.offset + (126 * n0 + 1) * W,
                            ap=[[W, 126], [126 * W, nblk], [1, W]])
                        nc.scalar.dma_start(
                            out_view,
                            ot[1:127, :].rearrange("p (n w) -> p n w",
                                                   n=nblk))
                        n0 += nblk

                    r0 = N_FULL_TILES * ROWS_PER_TILE + 1  # 1009
                    xt = xp.tile([32, W], f32, tag="xt_last")
                    nc.sync.dma_start(xt[0:16, :], x[s, H - 16:H, :])
                    nc.sync.dma_start(xt[16:32, :], x[s, 0:16, :])
                    pt = pp.tile([32, W], f32, tag="pt")
                    stencil_tile(xt[:], pt, 32)
                    ot = op.tile([32, W], f32, tag="ot")
                    nc.vector.scalar_tensor_tensor(
                        ot[:], xt[:], c2t[0:32, :], pt[:], op0=mult, op1=add)
                    nc.scalar.dma_start(y[s, r0:H, :], ot[1:1 + H - r0, :])
                    nc.scalar.dma_start(y[s, 0:1, :], ot[16:17, :])

            if loop_r == 1:
                body()
            else:
                with tc.For_i(0, loop_r, 1):
                    body()

    nc.compile()
    nc.m = get_hw_module(nc.m)
    return nc


def _kernel_exact(xs32, scale, in_dtype):
    from concourse.bass_utils import run_bass_kernel_spmd

    c1 = scale / 12.0
    c2 = 1.0 - 4.0 * scale / 3.0
    tri = np.zeros((128, 128), dtype=np.float64)
    idx = np.arange(128)
    tri[idx, idx] = 2.0
    tri[idx[:-1], idx[:-1] + 1] = 1.0
    tri[idx[:-1] + 1, idx[:-1]] = 1.0
    nc = _get_program("exact")
    w1 = (c1 * tri).astype(np.float32)
    w2 = (2.0 * c1 * tri).astype(np.float32)
    c2v = np.full((128, 1), c2, dtype=np.float32)
    in_maps = [
        {"x": xs32[k * S_PER_CORE:(k + 1) * S_PER_CORE], "w1": w1, "w2": w2,
         "c2v": c2v}
        for k in range(N_CORES)
    ]
    res = run_bass_kernel_spmd(nc, in_maps, core_ids=list(range(N_CORES)))
    out = np.concatenate([res.results[k]["y"] for k in range(N_CORES)], axis=0)
    return out.reshape(4, 8, H, W).astype(in_dtype, copy=False)
